# revision 1
# baseline (speedup 1.0000x reference)
"""Trainium2 Bass kernel for nn_ChebySemi (Chebyshev semi-iteration with
per-sample 3x3 stencil conv + power iteration), data-parallel over 8 cores.

Algorithm per sample (matches reference.py):
  power: 20x { y = conv3x3(pad(u)); m = max|y|; u = y/m }   -> m
  taus[k] = (1/m) * 2/(1.5 + 0.5*root_k)
  cheb:  15x { x += tau_k*(f - conv3x3(pad(x))) }

Mapping per core (8 samples):
  - padded image P [514,514] stored as 5 row-chunks [128 part, 5, 514] per
    sample; chunk c holds P rows [126c, 126c+128) (c<4), chunk 4 holds
    P rows [504,514) on partitions 0..9.
  - conv = banded matmuls (float32r): for each chunk, 3 col-shifted matmuls
    accumulate in PSUM; band S[k,p] = K[k-p+1,b] (k-p in {-1,0,1}, col p=0
    zeroed) so psum partition p aligns with U partition p.
  - per-step normalization: DVE abs-max reduces + cross-partition max via
    small transpose DMAs; ACT does the scaled PSUM->SBUF copy.
  - cheb update: ACT z=tau*f (bf16), GPSIMD U+=z, DVE U+=(-tau)*psum.
  - halo rows between chunks refreshed by 2 SBUF-SBUF DMAs per conv.
"""
import numpy as np
import ml_dtypes

B = 64
NCORES = 8
SPC = B // NCORES          # samples per core
M = 512
PW = 514
CH = 5                     # row chunks (4 main + 1 tail)
MMO = 127                  # matmul M (psum partitions; col 0 of band zeroed)
SW = CH * PW               # per-sample free width in U/F
NPOW = 20
NCHEB = 15
ALPHA = 0.5
ROOTS = np.cos(np.pi * (2 * np.arange(NCHEB) + 1) / (2 * NCHEB)).astype(np.float64)

_COMPILED = None


def _quant11(x):
    """Round fp32 to 11-bit mantissa (float32r input rounding)."""
    xi = np.ascontiguousarray(x, np.float32).view(np.uint32)
    shift = 23 - 11
    rb = np.uint32(1 << (shift - 1))
    mask = np.uint32(~((1 << shift) - 1) & 0xFFFFFFFF)
    return ((xi + rb) & mask).view(np.float32)


def _pad_layout(imgs, ones_pad):
    """imgs [N, 512, 512] -> [N, 128, 5, 514] chunk layout of padded P."""
    n = imgs.shape[0]
    P = np.zeros((n, PW, PW), np.float32)
    P[:, 1:513, 1:513] = imgs
    if ones_pad:
        P[:, 513, :] = 1.0
        P[:, :, 513] = 1.0
    out = np.zeros((n, 128, CH, PW), np.float32)
    for c in range(4):
        out[:, :, c, :] = P[:, 126 * c:126 * c + 128, :]
    out[:, 0:10, 4, :] = P[:, 504:514, :]
    return out


def _bands(kern):
    """kern [N,3,3] -> [N, 128, 3*127] shifted bands, col p=0 zeroed."""
    n = kern.shape[0]
    S = np.zeros((n, 128, 3 * MMO), np.float32)
    for b in range(3):
        for a in range(3):
            # S[:, k, b*127+p] = K[a, b] where k = p - 1 + a, p in [1,127)
            p = np.arange(1, MMO)
            k = p - 1 + a
            ok = (k >= 0) & (k < 128)
            S[:, k[ok], b * MMO + p[ok]] = kern[:, a, b][:, None]
    return S


def _build_program():
    import concourse.bass as bass
    import concourse.tile as tile
    from concourse import mybir, bacc
    from contextlib import ExitStack

    F32 = mybir.dt.float32
    F32R = mybir.dt.float32r
    BF16 = mybir.dt.bfloat16
    AX = mybir.AxisListType
    OP = mybir.AluOpType

    nc = bacc.Bacc("TRN2", target_bir_lowering=False, debug=False)

    u0p_d = nc.dram_tensor("u0p", [128, SPC * SW], F32, kind="ExternalInput")
    xp_d = nc.dram_tensor("xp", [128, SPC * SW], F32, kind="ExternalInput")
    fp_d = nc.dram_tensor("fp", [128, SPC * SW], BF16, kind="ExternalInput")
    wb_d = nc.dram_tensor("wb", [128, SPC * 3 * MMO], F32, kind="ExternalInput")
    c2q_d = nc.dram_tensor("c2q", [128, NCHEB], F32, kind="ExternalInput")
    nc2q_d = nc.dram_tensor("nc2q", [128, NCHEB], F32, kind="ExternalInput")
    out_d = nc.dram_tensor("out", [SPC * M, M], F32, kind="ExternalOutput")

    with tile.TileContext(nc) as tc, ExitStack() as ctx:
        sb = ctx.enter_context(tc.tile_pool(name="sb", bufs=1))
        ps = ctx.enter_context(tc.tile_pool(name="ps", bufs=3, space="PSUM"))
        p4p = ctx.enter_context(tc.tile_pool(name="p4p", bufs=2, space="PSUM"))
        zp = ctx.enter_context(tc.tile_pool(name="zp", bufs=2))

        U = sb.tile([128, SPC * SW], F32)
        Fm = sb.tile([128, SPC * SW], BF16)
        W = sb.tile([128, SPC * 3 * MMO], F32)
        C2Q = sb.tile([128, NCHEB], F32)
        NC2Q = sb.tile([128, NCHEB], F32)
        TAU = sb.tile([128, SPC * NCHEB], F32)
        NTAU = sb.tile([128, SPC * NCHEB], F32)
        RED3 = sb.tile([128, SPC * 4], F32)
        REDT = sb.tile([1, SPC * 128], F32)
        M1 = sb.tile([1, SPC], F32)
        INV1 = sb.tile([1, SPC], F32)
        INVROW = sb.tile([1, SPC * 128], F32)
        INVB = sb.tile([128, SPC], F32)
        ONES1 = sb.tile([1, 128], F32)

        nc.sync.dma_start(U[:].bitcast(F32R), u0p_d.ap()[:, :].bitcast(F32R))
        nc.sync.dma_start(Fm[:], fp_d.ap()[:, :])
        nc.sync.dma_start(W[:].bitcast(F32R), wb_d.ap()[:, :].bitcast(F32R))
        nc.sync.dma_start(C2Q[:], c2q_d.ap()[:, :])
        nc.sync.dma_start(NC2Q[:], nc2q_d.ap()[:, :])
        nc.vector.memset(ONES1[0:1, :], 1.0)
        nc.vector.memset(RED3[:, :], 0.0)

        def us(s):
            return U[:, s * SW:(s + 1) * SW].rearrange("p (c w) -> p c w", c=CH)

        def conv(s):
            Us = us(s)
            Ws = W[:, s * 3 * MMO:(s + 1) * 3 * MMO]
            pm0 = ps.tile([128, 1024], F32, tag="pm")
            pm1 = ps.tile([128, 1024], F32, tag="pm")
            p4 = p4p.tile([128, 512], F32, tag="p4")
            for g, pt in ((0, pm0), (1, pm1)):
                for ci in range(2):
                    c = 2 * g + ci
                    for b in range(3):
                        nc.tensor.matmul(
                            pt[0:MMO, ci * 512:(ci + 1) * 512],
                            Ws[:, b * MMO:(b + 1) * MMO].bitcast(F32R),
                            Us[0:128, c, b:b + 512].bitcast(F32R),
                            start=(b == 0), stop=(b == 2))
            for b in range(3):
                nc.tensor.matmul(
                    p4[0:9, 0:512],
                    Ws[0:10, b * MMO:b * MMO + 9].bitcast(F32R),
                    Us[0:10, 4, b:b + 512].bitcast(F32R),
                    start=(b == 0), stop=(b == 2))
            return pm0, pm1, p4

        def reduce_inv(s, pm0, pm1, p4):
            c0 = 4 * s
            nc.vector.tensor_reduce(
                RED3[0:MMO, c0:c0 + 1],
                pm0[0:MMO, :].rearrange("p (c w) -> p c w", c=2),
                axis=AX.XY, op=OP.max, apply_absolute_value=True)
            nc.vector.tensor_reduce(
                RED3[0:MMO, c0 + 1:c0 + 2],
                pm1[0:MMO, :].rearrange("p (c w) -> p c w", c=2),
                axis=AX.XY, op=OP.max, apply_absolute_value=True)
            nc.vector.tensor_reduce(
                RED3[0:9, c0 + 2:c0 + 3], p4[0:9, 0:512],
                axis=AX.X, op=OP.max, apply_absolute_value=True)
            nc.vector.tensor_reduce(
                RED3[0:128, c0 + 3:c0 + 4], RED3[0:128, c0:c0 + 3],
                axis=AX.X, op=OP.max)
            nc.sync.dma_start(REDT[0:1, s * 128:(s + 1) * 128],
                              RED3[0:128, c0 + 3:c0 + 4])
            nc.vector.tensor_reduce(
                M1[0:1, s:s + 1], REDT[0:1, s * 128:(s + 1) * 128],
                axis=AX.X, op=OP.max)
            nc.vector.reciprocal(INV1[0:1, s:s + 1], M1[0:1, s:s + 1])
            nc.vector.tensor_scalar_mul(
                INVROW[0:1, s * 128:(s + 1) * 128], ONES1[0:1, :],
                INV1[0:1, s:s + 1])
            nc.sync.dma_start(INVB[0:128, s:s + 1],
                              INVROW[0:1, s * 128:(s + 1) * 128])

        def halos(s):
            Us = us(s)
            nc.sync.dma_start(Us[0:1, 1:5, :].bitcast(F32R),
                              Us[126:127, 0:4, :].bitcast(F32R))
            nc.sync.dma_start(Us[127:128, 0:4, :].bitcast(F32R),
                              Us[1:2, 1:5, :].bitcast(F32R))

        # ---- power phase ----
        for it in range(1, NPOW + 1):
            for s in range(SPC):
                pm0, pm1, p4 = conv(s)
                reduce_inv(s, pm0, pm1, p4)
                if it < NPOW:
                    Us = us(s)
                    for g, pt in ((0, pm0), (1, pm1)):
                        nc.scalar.mul(
                            Us[0:MMO, 2 * g:2 * g + 2, 1:513].bitcast(F32R),
                            pt[0:MMO, :].rearrange("p (c w) -> p c w", c=2),
                            INVB[0:MMO, s:s + 1])
                    nc.scalar.mul(Us[0:9, 4, 1:513].bitcast(F32R),
                                  p4[0:9, 0:512], INVB[0:9, s:s + 1])
                    halos(s)
                else:
                    nc.vector.tensor_scalar_mul(
                        TAU[:, s * NCHEB:(s + 1) * NCHEB], C2Q[:, :],
                        INVB[:, s:s + 1])
                    nc.vector.tensor_scalar_mul(
                        NTAU[:, s * NCHEB:(s + 1) * NCHEB], NC2Q[:, :],
                        INVB[:, s:s + 1])

        # ---- cheb phase ----
        nc.sync.dma_start(U[:].bitcast(F32R), xp_d.ap()[:, :].bitcast(F32R))
        from concourse import mybir as _mb
        for k in range(NCHEB):
            for s in range(SPC):
                pm0, pm1, p4 = conv(s)
                Us = us(s)
                z = zp.tile([128, SW], BF16, tag="z")
                nc.scalar.mul(z[:, :], Fm[:, s * SW:(s + 1) * SW],
                              TAU[:, s * NCHEB + k:s * NCHEB + k + 1])
                nc.gpsimd.tensor_tensor(
                    U[:, s * SW:(s + 1) * SW].bitcast(F32R),
                    U[:, s * SW:(s + 1) * SW], z[:, :], op=OP.add)
                for g, pt in ((0, pm0), (1, pm1)):
                    nc.vector.scalar_tensor_tensor(
                        Us[0:MMO, 2 * g:2 * g + 2, 1:513].bitcast(F32R),
                        pt[0:MMO, :].rearrange("p (c w) -> p c w", c=2),
                        NTAU[0:MMO, s * NCHEB + k:s * NCHEB + k + 1],
                        Us[0:MMO, 2 * g:2 * g + 2, 1:513],
                        op0=OP.mult, op1=OP.add)
                nc.vector.scalar_tensor_tensor(
                    Us[0:9, 4, 1:513].bitcast(F32R), p4[0:9, 0:512],
                    NTAU[0:9, s * NCHEB + k:s * NCHEB + k + 1],
                    Us[0:9, 4, 1:513], op0=OP.mult, op1=OP.add)
                if k < NCHEB - 1:
                    halos(s)

        for s in range(SPC):
            Us = us(s)
            o = out_d.ap()[s * M:(s + 1) * M, :]
            nc.sync.dma_start(
                o[0:504, :].rearrange("(c p) w -> p c w", p=126),
                Us[1:MMO, 0:4, 1:513])
            nc.sync.dma_start(o[504:512, :], Us[1:9, 4, 1:513])

    nc.compile()
    return nc


def _prep_core_inputs(x, f, kernelA, u0):
    """Full [64,...] inputs -> list of 8 per-core input dicts."""
    x = np.asarray(x, np.float32).reshape(B, M, M)
    f = np.asarray(f, np.float32).reshape(B, M, M)
    kern = np.asarray(kernelA, np.float32).reshape(B, 3, 3)
    u0 = np.asarray(u0, np.float32).reshape(B, M, M)

    u0L = _quant11(_pad_layout(u0, True))     # [B,128,CH,PW]
    xL = _quant11(_pad_layout(x, True))
    fL = _pad_layout(f, False).astype(ml_dtypes.bfloat16)
    wbL = _quant11(_bands(kern))              # [B,128,381]

    c2q = (2.0 / (1.5 + 0.5 * ROOTS)).astype(np.float32)
    c2qT = np.broadcast_to(c2q, (128, NCHEB)).copy()
    nc2qT = (-c2qT).copy()

    in_maps = []
    for c in range(NCORES):
        sl = slice(c * SPC, (c + 1) * SPC)
        in_maps.append({
            "u0p": u0L[sl].transpose(1, 0, 2, 3).reshape(128, SPC * SW).copy(),
            "xp": xL[sl].transpose(1, 0, 2, 3).reshape(128, SPC * SW).copy(),
            "fp": fL[sl].transpose(1, 0, 2, 3).reshape(128, SPC * SW).copy(),
            "wb": wbL[sl].transpose(1, 0, 2).reshape(128, SPC * 3 * MMO).copy(),
            "c2q": c2qT,
            "nc2q": nc2qT,
        })
    return in_maps


def kernel(x, f, kernelA, u0):
    global _COMPILED
    from concourse import bass_utils

    if _COMPILED is None:
        _COMPILED = _build_program()
    nc = _COMPILED

    in_maps = _prep_core_inputs(x, f, kernelA, u0)
    res = bass_utils.run_bass_kernel_spmd(nc, in_maps, core_ids=list(range(NCORES)))
    out = np.stack([res.results[c]["out"] for c in range(NCORES)])  # [8, SPC*M, M]
    return out.reshape(B, 1, M, M).astype(np.float32)



# revision 25
# speedup vs baseline: 1.8950x; 1.8950x over previous
"""Trainium2 Bass kernel for nn_ChebySemi (Chebyshev semi-iteration with
per-sample 3x3 stencil conv + power iteration), data-parallel over 8 cores.

Algorithm per sample (matches reference.py):
  power: 20x { y = conv3x3(pad(u)); m = max|y|; u = y/m }   -> m
  taus[k] = (1/m) * 2/(1.5 + 0.5*root_k)
  cheb:  15x { x += tau_k*(f - conv3x3(pad(x))) }

Key restructurings vs the naive mapping:
  - power runs UNNORMALIZED: with the ones-pad the operator is affine,
    y = A0 u + b1 (b1 = conv of the ones-ring, nonzero only on the last
    row/col). We iterate v_{k+1} = A0 v_k + s_k*b1 with s_k = max|v_k|,
    which satisfies v_k = (prod m_j) u_k exactly, so m = s20/s19.
    fp32 exponent absorbs the growth (|lambda| ~< 15, 15^20 << 1e38).
    The s_k*b1 term is injected into the conv's own PSUM accumulation
    by tiny rank-1 matmuls (contraction dim 1), so the per-iteration
    normalization (scaled ACT copy + broadcast) disappears from the
    critical path: s_k only feeds the NEXT iteration's fold matmuls.
  - cheb folds -f into the conv's PSUM accumulation with a shared
    identity-band matmul (bf16), so psum = A x - f and the update is a
    single DVE scalar_tensor_tensor: U += (-tau) * psum.

Mapping per core (8 samples):
  - padded image P [514,514] stored as 5 row-chunks [128 part, 5, 514] per
    sample; chunk c holds P rows [126c, 126c+128) (c<4), chunk 4 holds
    P rows [504,514) on partitions 0..9.
  - conv = banded matmuls (float32r): for each chunk, 3 col-shifted matmuls
    accumulate in PSUM; band S[k,p] = K[k-p+1,b] (k-p in {-1,0,1}, col p=0
    zeroed) so psum partition p aligns with U partition p.
  - halo rows between chunks refreshed by 2 SBUF-SBUF DMAs per conv.
"""
import numpy as np
import ml_dtypes

B = 64
NCORES = 8
SPC = B // NCORES          # samples per core
M = 512
PW = 514
CH = 5                     # row chunks (4 main + 1 tail)
MMO = 127                  # matmul M (psum partitions; col 0 of band zeroed)
SW = CH * PW               # per-sample free width in U/F
NPOW = 20
NCHEB = 15
ALPHA = 0.5
ROOTS = np.cos(np.pi * (2 * np.arange(NCHEB) + 1) / (2 * NCHEB)).astype(np.float64)

_COMPILED = None


def _quant11(x):
    """Round fp32 to 11-bit mantissa (float32r input rounding)."""
    xi = np.ascontiguousarray(x, np.float32).view(np.uint32)
    shift = 23 - 11
    rb = np.uint32(1 << (shift - 1))
    mask = np.uint32(~((1 << shift) - 1) & 0xFFFFFFFF)
    return ((xi + rb) & mask).view(np.float32)


def _pad_layout(imgs, ones_pad):
    """imgs [N, 512, 512] -> [N, 128, 5, 514] chunk layout of padded P."""
    n = imgs.shape[0]
    P = np.zeros((n, PW, PW), np.float32)
    P[:, 1:513, 1:513] = imgs
    if ones_pad:
        P[:, 513, :] = 1.0
        P[:, :, 513] = 1.0
    out = np.zeros((n, 128, CH, PW), np.float32)
    for c in range(4):
        out[:, :, c, :] = P[:, 126 * c:126 * c + 128, :]
    out[:, 0:10, 4, :] = P[:, 504:514, :]
    return out


def _bands(kern):
    """kern [N,3,3] -> [N, 128, 3*127] shifted bands, col p=0 zeroed."""
    n = kern.shape[0]
    S = np.zeros((n, 128, 3 * MMO), np.float32)
    for b in range(3):
        for a in range(3):
            # S[:, k, b*127+p] = K[a, b] where k = p - 1 + a, p in [1,127)
            p = np.arange(1, MMO)
            k = p - 1 + a
            ok = (k >= 0) & (k < 128)
            S[:, k[ok], b * MMO + p[ok]] = kern[:, a, b][:, None]
    return S


def _ident_bands():
    """Negated identity bands for the -f fold: main [128,127], tail [10,9].
    Packed as one [128, 136] bf16 array (tail in cols 127:136, rows 0:10)."""
    ib = np.zeros((128, 136), np.float32)
    d = np.arange(1, MMO)
    ib[d, d] = -1.0                       # main: p = 1..126, k == p
    dt = np.arange(0, 9)
    ib[dt, MMO + dt] = -1.0               # tail: p = 0..8, k == p
    return ib.astype(ml_dtypes.bfloat16)


# boundary-fold vector layout (per sample, on partition 0):
#   [0:508)    CV: b1[:,511] per main chunk c, index c*127+p (p=1..126)
#   [508:517)  CV4: b1[503+p, 511], p=0..8
#   [517:1029) B1R: b1[511, 0:512] with corner zeroed
BFW = 4 * MMO + 9 + 512
# trailing constants after the SPC per-sample blocks:
#   [SPC*BFW : +9)            RV row-fold indicator (1.0 at p=8)
#   [SPC*BFW+16 : +2*SPC)     SVZ init: (0, 1) pairs per sample
BFTOT = SPC * BFW + 16 + 2 * SPC


def _bound_fold(kern):
    """kern [N,3,3] -> [N, 1029] boundary-fold vectors (see layout above)."""
    n = kern.shape[0]
    ring = np.zeros((PW, PW), np.float32)
    ring[513, :] = 1.0
    ring[:, 513] = 1.0
    bf = np.zeros((n, BFW), np.float32)
    for i in range(n):
        b1 = np.zeros((M, M), np.float32)
        for a in range(3):
            for b in range(3):
                b1 += kern[i, a, b] * ring[a:a + M, b:b + M]
        for c in range(4):
            p = np.arange(1, MMO)
            bf[i, c * MMO + p] = b1[126 * c + p - 1, 511]
        p = np.arange(0, 9)
        bf[i, 4 * MMO + p] = b1[503 + p, 511]
        bf[i, 4 * MMO + 9:4 * MMO + 9 + 511] = b1[511, 0:511]
    return bf


def _bf_row(bfL):
    """[SPC,1029] per-core fold data -> full [1, BFTOT] row with consts."""
    row = np.zeros((1, BFTOT), np.float32)
    row[0, :SPC * BFW] = bfL.reshape(-1)
    row[0, SPC * BFW + 8] = 1.0                   # RV indicator at p=8
    row[0, SPC * BFW + 16 + 1::2] = 1.0           # SVZ init s=1.0
    return row


def _build_program():
    import concourse.bass as bass
    import concourse.tile as tile
    from concourse import mybir, bacc
    from contextlib import ExitStack

    F32 = mybir.dt.float32
    F32R = mybir.dt.float32r
    BF16 = mybir.dt.bfloat16
    AX = mybir.AxisListType
    OP = mybir.AluOpType

    nc = bacc.Bacc("TRN2", target_bir_lowering=False, debug=False)

    u0p_d = nc.dram_tensor("u0p", [128, SPC * SW], F32, kind="ExternalInput")
    xp_d = nc.dram_tensor("xp", [128, SPC * SW], F32, kind="ExternalInput")
    fp_d = nc.dram_tensor("fp", [128, SPC * SW], BF16, kind="ExternalInput")
    wb_d = nc.dram_tensor("wb", [128, SPC * 3 * MMO], F32, kind="ExternalInput")
    ib_d = nc.dram_tensor("ib", [128, 136], BF16, kind="ExternalInput")
    bf_d = nc.dram_tensor("bf", [1, BFTOT], F32, kind="ExternalInput")
    nc2q_d = nc.dram_tensor("nc2q", [128, NCHEB], F32, kind="ExternalInput")
    out_d = nc.dram_tensor("out", [SPC * M, M], F32, kind="ExternalOutput")

    with tile.TileContext(nc) as tc, ExitStack() as ctx:
        sb = ctx.enter_context(tc.tile_pool(name="sb", bufs=1))
        ps = ctx.enter_context(tc.tile_pool(name="ps", bufs=3, space="PSUM"))
        p4p = ctx.enter_context(tc.tile_pool(name="p4p", bufs=2, space="PSUM"))

        U = sb.tile([128, SPC * SW], F32)
        Fm = sb.tile([128, SPC * SW], BF16)
        W = sb.tile([128, SPC * 3 * MMO], F32)
        IB = sb.tile([128, 136], BF16)
        BF = sb.tile([1, BFTOT], F32)
        NC2Q = sb.tile([128, NCHEB], F32)
        NTAU = sb.tile([128, SPC * NCHEB], F32)
        RED = sb.tile([128, SPC * 8], F32)
        REDT = sb.tile([1, SPC * 256], F32)
        S19 = sb.tile([1, SPC], F32)
        S20 = sb.tile([1, SPC], F32)
        SR2 = sb.tile([1, SPC * 512], F32)
        INV1 = sb.tile([1, SPC], F32)
        RAT = sb.tile([1, SPC], F32)
        RROW = sb.tile([1, SPC * 128], F32)
        RB = sb.tile([128, SPC], F32)
        ONES1 = sb.tile([1, 128], F32)

        for s in range(SPC):
            nc.sync.dma_start(U[:, s * SW:(s + 1) * SW].bitcast(F32R),
                              u0p_d.ap()[:, s * SW:(s + 1) * SW].bitcast(F32R))
            nc.sync.dma_start(
                W[:, s * 3 * MMO:(s + 1) * 3 * MMO].bitcast(F32R),
                wb_d.ap()[:, s * 3 * MMO:(s + 1) * 3 * MMO].bitcast(F32R))
            nc.sync.dma_start(Fm[:, s * SW:(s + 1) * SW],
                              fp_d.ap()[:, s * SW:(s + 1) * SW])
        nc.sync.dma_start(IB[:], ib_d.ap()[:, :])
        nc.sync.dma_start(BF[:].bitcast(F32R), bf_d.ap()[:, :].bitcast(F32R))
        nc.sync.dma_start(NC2Q[:], nc2q_d.ap()[:, :])
        nc.vector.memset(ONES1[0:1, :], 1.0)
        nc.vector.memset(RED[:, :], 0.0)

        SVBASE = SPC * BFW + 16

        def RVap():
            return BF[0:1, SPC * BFW:SPC * BFW + 9]

        def SVpair(s):
            return BF[0:1, SVBASE + 2 * s:SVBASE + 2 * s + 2]

        def SVs(s):
            return BF[0:1, SVBASE + 2 * s + 1:SVBASE + 2 * s + 2]

        def us(s):
            return U[:, s * SW:(s + 1) * SW].rearrange("p (c w) -> p c w", c=CH)

        def fs(s):
            return Fm[:, s * SW:(s + 1) * SW].rearrange("p (c w) -> p c w", c=CH)

        def BFs_row(s):
            o = s * BFW + 4 * MMO + 9
            return BF[0:1, o:o + 512]

        def conv(s, fold_f=False, fold_b1=False):
            """3x3 banded conv of sample s; optionally accumulate -f
            (cheb) or the s_k-scaled boundary term (power)."""
            Us = us(s)
            Ws = W[:, s * 3 * MMO:(s + 1) * 3 * MMO]
            BFs = BF[0:1, s * BFW:(s + 1) * BFW]
            pm0 = ps.tile([128, 1024], F32, tag="pm")
            pm1 = ps.tile([128, 1024], F32, tag="pm")
            p4 = p4p.tile([128, 512], F32, tag="p4")
            extra = fold_f or fold_b1
            for g, pt in ((0, pm0), (1, pm1)):
                for ci in range(2):
                    c = 2 * g + ci
                    for b in range(3):
                        nc.tensor.matmul(
                            pt[0:MMO, ci * 512:(ci + 1) * 512],
                            Ws[:, b * MMO:(b + 1) * MMO].bitcast(F32R),
                            Us[0:128, c, b:b + 512].bitcast(F32R),
                            start=(b == 0), stop=(b == 2 and not extra))
                    if fold_f:
                        nc.tensor.matmul(
                            pt[0:MMO, ci * 512:(ci + 1) * 512],
                            IB[:, 0:MMO],
                            fs(s)[0:128, c, 1:513],
                            start=False, stop=True)
                    if fold_b1:
                        # psum[p, 510:512] += b1[row(p), 511] * (0, s_prev)
                        nc.tensor.matmul(
                            pt[0:MMO, ci * 512 + 510:ci * 512 + 512],
                            BFs[0:1, c * MMO:(c + 1) * MMO].bitcast(F32R),
                            SVpair(s).bitcast(F32R),
                            start=False, stop=True)
            for b in range(3):
                nc.tensor.matmul(
                    p4[0:9, 0:512],
                    Ws[0:10, b * MMO:b * MMO + 9].bitcast(F32R),
                    Us[0:10, 4, b:b + 512].bitcast(F32R),
                    start=(b == 0), stop=(b == 2 and not extra))
            if fold_f:
                nc.tensor.matmul(
                    p4[0:9, 0:512],
                    IB[0:10, MMO:MMO + 9],
                    fs(s)[0:10, 4, 1:513],
                    start=False, stop=True)
            if fold_b1:
                nc.tensor.matmul(
                    p4[0:9, 510:512],
                    BFs[0:1, 4 * MMO:4 * MMO + 9].bitcast(F32R),
                    SVpair(s).bitcast(F32R),
                    start=False, stop=False)
                # psum[8, :] += b1[511, :] * s_prev (via indicator lhsT)
                nc.tensor.matmul(
                    p4[0:9, 0:512],
                    RVap().bitcast(F32R),
                    SR2[0:1, s * 512:(s + 1) * 512].bitcast(F32R),
                    start=False, stop=True)
            return pm0, pm1, p4

        def reduce_max(s, it, pm0, pm1, p4):
            """abs-max of psum pieces -> SV[0:1, s:s+1] (+S19/S20)."""
            ph = it % 2
            c0 = 8 * s + 4 * ph
            nc.vector.tensor_reduce(
                RED[0:MMO, c0:c0 + 1],
                pm0[0:MMO, :].rearrange("p (c w) -> p c w", c=2),
                axis=AX.XY, op=OP.max, apply_absolute_value=True)
            nc.vector.tensor_reduce(
                RED[0:MMO, c0 + 1:c0 + 2],
                pm1[0:MMO, :].rearrange("p (c w) -> p c w", c=2),
                axis=AX.XY, op=OP.max, apply_absolute_value=True)
            nc.vector.tensor_reduce(
                RED[0:9, c0 + 2:c0 + 3], p4[0:9, 0:512],
                axis=AX.X, op=OP.max, apply_absolute_value=True)
            nc.vector.tensor_reduce(
                RED[0:128, c0 + 3:c0 + 4], RED[0:128, c0:c0 + 3],
                axis=AX.X, op=OP.max)
            t0 = s * 256 + ph * 128
            nc.sync.dma_start(REDT[0:1, t0:t0 + 128], RED[0:128, c0 + 3:c0 + 4])
            if it == NPOW - 1:
                nc.vector.tensor_reduce(
                    S19[0:1, s:s + 1], REDT[0:1, t0:t0 + 128],
                    axis=AX.X, op=OP.max)
            if it == NPOW:
                nc.vector.tensor_reduce(
                    S20[0:1, s:s + 1], REDT[0:1, t0:t0 + 128],
                    axis=AX.X, op=OP.max)
            else:
                nc.vector.tensor_reduce(
                    SVs(s).bitcast(F32R), REDT[0:1, t0:t0 + 128],
                    axis=AX.X, op=OP.max)

        def halos(s):
            Us = us(s)
            nc.sync.dma_start(Us[0:1, 1:5, :].bitcast(F32R),
                              Us[126:127, 0:4, :].bitcast(F32R))
            nc.sync.dma_start(Us[127:128, 0:4, :].bitcast(F32R),
                              Us[1:2, 1:5, :].bitcast(F32R))

        # ---- power phase: v_{k+1} = A0 v_k + s_k b1, m = s20/s19 ----
        for it in range(1, NPOW + 1):
            for s in range(SPC):
                # SR2 = s_prev * b1[511, :] for the row fold (ACT, off path)
                nc.scalar.mul(SR2[0:1, s * 512:(s + 1) * 512].bitcast(F32R),
                              BFs_row(s), SVs(s))
                pm0, pm1, p4 = conv(s, fold_b1=True)
                reduce_max(s, it, pm0, pm1, p4)
                if it < NPOW:
                    Us = us(s)
                    for g, pt in ((0, pm0), (1, pm1)):
                        nc.scalar.copy(
                            Us[0:MMO, 2 * g:2 * g + 2, 1:513].bitcast(F32R),
                            pt[0:MMO, :].rearrange("p (c w) -> p c w", c=2))
                    nc.scalar.copy(Us[0:9, 4, 1:513].bitcast(F32R),
                                   p4[0:9, 0:512])
                    halos(s)
                else:
                    # 1/m = s19/s20; NTAU = nc2q * (1/m)
                    nc.vector.reciprocal(INV1[0:1, s:s + 1], S20[0:1, s:s + 1])
                    nc.vector.tensor_tensor(
                        RAT[0:1, s:s + 1], S19[0:1, s:s + 1],
                        INV1[0:1, s:s + 1], op=OP.mult)
                    nc.vector.tensor_scalar_mul(
                        RROW[0:1, s * 128:(s + 1) * 128], ONES1[0:1, :],
                        RAT[0:1, s:s + 1])
                    nc.sync.dma_start(RB[0:128, s:s + 1],
                                      RROW[0:1, s * 128:(s + 1) * 128])
                    nc.vector.tensor_scalar_mul(
                        NTAU[:, s * NCHEB:(s + 1) * NCHEB], NC2Q[:, :],
                        RB[:, s:s + 1])
                    # load x into U for the cheb phase
                    nc.sync.dma_start(
                        U[:, s * SW:(s + 1) * SW].bitcast(F32R),
                        xp_d.ap()[:, s * SW:(s + 1) * SW].bitcast(F32R))

        # ---- cheb phase: U += (-tau_k) * (A x - f) ----
        for k in range(NCHEB):
            for s in range(SPC):
                pm0, pm1, p4 = conv(s, fold_f=True)
                Us = us(s)
                for g, pt in ((0, pm0), (1, pm1)):
                    nc.vector.scalar_tensor_tensor(
                        Us[0:MMO, 2 * g:2 * g + 2, 1:513].bitcast(F32R),
                        pt[0:MMO, :].rearrange("p (c w) -> p c w", c=2),
                        NTAU[0:MMO, s * NCHEB + k:s * NCHEB + k + 1],
                        Us[0:MMO, 2 * g:2 * g + 2, 1:513],
                        op0=OP.mult, op1=OP.add)
                nc.vector.scalar_tensor_tensor(
                    Us[0:9, 4, 1:513].bitcast(F32R), p4[0:9, 0:512],
                    NTAU[0:9, s * NCHEB + k:s * NCHEB + k + 1],
                    Us[0:9, 4, 1:513], op0=OP.mult, op1=OP.add)
                if k < NCHEB - 1:
                    halos(s)

        for s in range(SPC):
            Us = us(s)
            o = out_d.ap()[s * M:(s + 1) * M, :]
            nc.sync.dma_start(
                o[0:504, :].rearrange("(c p) w -> p c w", p=126),
                Us[1:MMO, 0:4, 1:513])
            nc.sync.dma_start(o[504:512, :], Us[1:9, 4, 1:513])

    nc.compile()
    return nc


def _prep_core_inputs(x, f, kernelA, u0):
    """Full [64,...] inputs -> list of 8 per-core input dicts."""
    x = np.asarray(x, np.float32).reshape(B, M, M)
    f = np.asarray(f, np.float32).reshape(B, M, M)
    kern = np.asarray(kernelA, np.float32).reshape(B, 3, 3)
    u0 = np.asarray(u0, np.float32).reshape(B, M, M)

    u0L = _quant11(_pad_layout(u0, False))    # [B,128,CH,PW] zero pads
    xL = _quant11(_pad_layout(x, True))
    fL = _pad_layout(f, False).astype(ml_dtypes.bfloat16)
    wbL = _quant11(_bands(kern))              # [B,128,381]
    ibL = _ident_bands()                      # [128,136] bf16
    bfL = _bound_fold(kern)                   # [B,1029]

    nc2q = (-2.0 / (1.5 + 0.5 * ROOTS)).astype(np.float32)
    nc2qT = np.broadcast_to(nc2q, (128, NCHEB)).copy()

    in_maps = []
    for c in range(NCORES):
        sl = slice(c * SPC, (c + 1) * SPC)
        in_maps.append({
            "u0p": u0L[sl].transpose(1, 0, 2, 3).reshape(128, SPC * SW).copy(),
            "xp": xL[sl].transpose(1, 0, 2, 3).reshape(128, SPC * SW).copy(),
            "fp": fL[sl].transpose(1, 0, 2, 3).reshape(128, SPC * SW).copy(),
            "wb": wbL[sl].transpose(1, 0, 2).reshape(128, SPC * 3 * MMO).copy(),
            "ib": ibL,
            "bf": _bf_row(bfL[sl]),
            "nc2q": nc2qT,
        })
    return in_maps


def kernel(x, f, kernelA, u0):
    global _COMPILED
    from concourse import bass_utils

    if _COMPILED is None:
        _COMPILED = _build_program()
    nc = _COMPILED

    in_maps = _prep_core_inputs(x, f, kernelA, u0)
    res = bass_utils.run_bass_kernel_spmd(nc, in_maps, core_ids=list(range(NCORES)))
    out = np.stack([res.results[c]["out"] for c in range(NCORES)])  # [8, SPC*M, M]
    return out.reshape(B, 1, M, M).astype(np.float32)
